# revision 1
# baseline (speedup 1.0000x reference)
"""Distributed causal multi-head attention for 8 TRN2 NeuronCores.

Problem: B=4, S=2048, D=1024, H=16 heads of DH=64, fp32, causal + padding mask.

Sharding: core c -> (batch b = c//2, head-group g = c%2 of 8 heads).
Each core computes, for its (b, g):
    QT = Wq_g @ X_q^T          (512, 2048)   [head dims on partitions]
    KT = Wk_g @ X_kv^T         (512, 2048)
    V  = X_kv @ Wv_g^T         (2048, 512)   [keys on partitions, +ones col per head]
    per head h: S^T = K_h Q_h^T             (keys on partitions, queries free)
                E = exp(S^T * scale + pad_bias), causal-masked
                Oaug^T = matmul(lhsT=V_aug_h, rhs=E) -> (65, q)
                  row 64 = softmax denominators (ones-column trick)
                attT[h] = Oaug^T[0:64] * (1/Oaug^T[64]) broadcast over partitions
    outT_partial = matmul(lhsT=woT, rhs=attT) -> (1024, 2048)
Host sums the two per-batch partials and transposes back.

All matmuls run as float32r. PSUM is organized as four (128,1024) two-bank
tiles A..D: the Q projection uses all four; K/V projections only A/B (split
into two 4-group passes) so the attention score tiles (C/D) are free as soon
as the Q projection retires -- the first head's scores+exp overlap the K/V
projections, keeping the PE activity window dense.
"""

import numpy as np

import concourse.bass as bass
import concourse.mybir as mybir
import concourse.tile as tile
from concourse import bacc

B, S, D, H = 4, 2048, 1024, 16
DH = 64
NG = 2              # head groups (cores per batch)
DG = D // NG        # 512 head dims per core
HL = H // NG        # 8 heads per core
PB = 128            # partition block
CH = 512            # free-dim chunk (one fp32 PSUM bank)
NCH = S // CH       # 4 chunks
NKT = S // PB       # 16 key tiles
NDT = D // PB       # 8 contraction tiles for projections
NJT = DG // PB      # 4 head-dim tiles per core
HS = S // 2         # 1024, half of seq
F32 = mybir.dt.float32
F32R = mybir.dt.float32r
F16 = mybir.dt.float16
SCALE = 1.0 / 8.0   # 1/sqrt(DH)


def _r(ap):
    return ap.bitcast(F32R)


def _emit(nc, xq, xkv, wq, wk, wv, wo, pb, outT):
    with tile.TileContext(nc) as tc:
        with (
            tc.tile_pool(name="pers", bufs=1) as pers,
            tc.tile_pool(name="big", bufs=1) as bigp,
            tc.tile_pool(name="qt", bufs=1) as qtp,
            tc.tile_pool(name="kt", bufs=1) as ktp,
            tc.tile_pool(name="vt", bufs=1) as vtp,
            tc.tile_pool(name="wp", bufs=1) as wp,
            tc.tile_pool(name="wo", bufs=1) as wop,
            tc.tile_pool(name="ex", bufs=2) as exp_pool,
            tc.tile_pool(name="stg", bufs=2) as stgp,
            tc.tile_pool(name="rc", bufs=4) as rcp,
            tc.tile_pool(name="ps", bufs=1, space="PSUM") as ps,
            tc.tile_pool(name="dram", bufs=1, space="DRAM") as dramp,
        ):
            # ---------------- persistent small tiles ----------------
            # padding bias laid out (128, 16): pbias_sb[p, i] = pb[i*128 + p]
            pbias_sb = pers.tile([PB, NKT], F32, tag="pbias", name="pbias_sb")
            nc.sync.dma_start(out=pbias_sb[:], in_=pb[:].rearrange("(i p) -> p i", p=PB))

            # ---------------- long-lived activation tiles ----------------
            qt = [qtp.tile([PB, S], F32R, tag=f"qt{j}", name=f"qt{j}") for j in range(NJT)]
            kt = [ktp.tile([PB, S], F32R, tag=f"kt{j}", name=f"kt{j}") for j in range(NJT)]
            # V with one extra "ones" column per head: (128, 8*65)
            vt = [vtp.tile([PB, HL * (DH + 1)], F16, tag=f"vt{i}", name=f"vt{i}") for i in range(NKT)]
            ones8 = pers.tile([PB, HL], F32, tag="ones8", name="ones8")
            nc.gpsimd.memset(ones8[:], 1.0)
            for i in range(NKT):
                ones_view = vt[i][:].rearrange("p (h c) -> p h c", c=DH + 1)[:, :, DH]
                nc.vector.tensor_copy(ones_view, ones8[:])

            attd = dramp.tile([DG, S], F32R, tag="attd", name="attd")

            # PSUM: four (128, 1024) two-bank tiles, tags A..D
            def pair_tile(tag):
                return ps.tile([PB, 2 * CH], F32, tag=tag, name=f"ps{tag}")

            def halves(t):
                return [t[:, 0:CH], t[:, CH:2 * CH]]

            def load_w(dram_w, d):
                t = wp.tile([PB, DG], F32R, tag=f"w{d}", name=f"w{d}")
                nc.sync.dma_start(out=t[:], in_=dram_w[d * PB:(d + 1) * PB, :])
                return t

            def load_xh(dram_x, d, half):
                t = bigp.tile([PB, HS], F32R, tag=f"b{d}", name=f"xh{d}")
                nc.sync.dma_start(
                    out=t[:], in_=dram_x[d * PB:(d + 1) * PB,
                                         half * HS:(half + 1) * HS])
                return t

            # ---------------- Q projection (8 groups on A..D) ----------------
            # emit every load up front: half-1 tile DMAs fire as soon as the
            # slot's half-0 tile retires (mid-loop), ahead of the K prefetch
            xq_halves = [[load_xh(xq, d, hf) for d in range(NDT)] for hf in range(2)]
            bx_pre = []
            for d in range(4):
                t = bigp.tile([PB, HS], F32R, tag=f"bx{d}", name=f"bx{d}")
                nc.sync.dma_start(out=t[:], in_=xkv[d * PB:(d + 1) * PB, 0:HS])
                bx_pre.append(t)
            for half in range(2):
                xh = xq_halves[half]
                accs = []
                for tag in "ABCD":
                    accs += halves(pair_tile(tag))
                wts = [load_w(wq, d) for d in range(NDT)]
                for d in range(NDT):
                    for j in range(NJT):
                        for ci in range(2):
                            nc.tensor.matmul(
                                accs[j * 2 + ci],
                                _r(wts[d][:, j * PB:(j + 1) * PB]),
                                _r(xh[d][:, ci * CH:(ci + 1) * CH]),
                                start=(d == 0), stop=(d == NDT - 1),
                            )
                for j in range(NJT):
                    for ci in range(2):
                        c = half * 2 + ci
                        nc.vector.tensor_copy(
                            qt[j][:, c * CH:(c + 1) * CH], accs[j * 2 + ci])

            # ------------- K/V projections (4-group passes on A/B) -------------
            for half in range(2):
                # weights first: K's opening matmuls need wk[0] + the bx
                # prefetch; the bulkier xkv loads can trail behind them
                wts = [load_w(wk, d) for d in range(NDT)]
                if half == 0:
                    xh = bx_pre + [load_xh(xkv, d, half) for d in range(4, NDT)]
                else:
                    xh = []
                    for d in range(NDT):
                        if d < 4:
                            t = bigp.tile([PB, HS], F32R, tag=f"bx{d}", name=f"bx{d}b")
                            nc.sync.dma_start(
                                out=t[:], in_=xkv[d * PB:(d + 1) * PB, HS:S])
                            xh.append(t)
                        else:
                            xh.append(load_xh(xkv, d, half))
                for jp in range(2):
                    accs = halves(pair_tile("A")) + halves(pair_tile("B"))
                    for d in range(NDT):
                        for jj in range(2):
                            j = jp * 2 + jj
                            for ci in range(2):
                                nc.tensor.matmul(
                                    accs[jj * 2 + ci],
                                    _r(wts[d][:, j * PB:(j + 1) * PB]),
                                    _r(xh[d][:, ci * CH:(ci + 1) * CH]),
                                    start=(d == 0), stop=(d == NDT - 1),
                                )
                    for jj in range(2):
                        j = jp * 2 + jj
                        for ci in range(2):
                            c = half * 2 + ci
                            nc.vector.tensor_copy(
                                kt[j][:, c * CH:(c + 1) * CH], accs[jj * 2 + ci])
                wvs = [load_w(wv, d) for d in range(NDT)]
                for sp in range(2):
                    accs = halves(pair_tile("A")) + halves(pair_tile("B"))
                    for d in range(NDT):
                        for s4 in range(4):
                            si = sp * 4 + s4
                            nc.tensor.matmul(
                                accs[s4],
                                _r(xh[d][:, si * PB:(si + 1) * PB]),
                                _r(wvs[d][:]),
                                start=(d == 0), stop=(d == NDT - 1),
                            )
                    for s4 in range(4):
                        i = half * 8 + sp * 4 + s4
                        src = accs[s4].rearrange("p (h c) -> p h c", c=DH)
                        dst = vt[i][:].rearrange("p (h c) -> p h c", c=DH + 1)[:, :, 0:DH]
                        nc.vector.tensor_copy(dst, src)

            # prefetch output-projection weights and stage the attT loads
            # early: each att_half row-block DMA fires as soon as its head
            # lands in DRAM, so the final head's data is the only tail wait
            wol = []
            for j in range(NJT):
                t = wop.tile([PB, D], F32R, tag=f"wo{j}", name=f"wo{j}")
                nc.sync.dma_start(out=t[:], in_=wo[j * PB:(j + 1) * PB, :])
                wol.append(t)

            # ---------------- attention, one head at a time ----------------
            # scores/exp run on C/D (free right after the Q projection);
            # AV accumulators pair chunks {0,1}->A, {2,3}->B (free after V).
            st_cnt = 0
            for h in range(HL):
                jq = h // 2
                rowo = (h % 2) * DH       # row offset inside the qt/kt tiles

                stg_t = stgp.tile([DH, S], F32R, tag="stg", name="stg_t")
                opair = [pair_tile("A"), pair_tile("B")]

                def oaug(c):
                    return opair[c // 2][:, (c % 2) * CH:(c % 2 + 1) * CH]

                for i in range(NKT):
                    c0 = i // 4                     # first valid (causal) chunk
                    ex_t = exp_pool.tile([PB, S], F16, tag="ex", bufs=3, name="ex_t")
                    for hh in range(c0 // 2, 2):    # q-halves holding valid chunks
                        st_t = pair_tile("CD"[st_cnt % 2])
                        st_cnt += 1
                        lo_c = max(c0, hh * 2)
                        for c in range(lo_c, hh * 2 + 2):
                            q_lo = max(c * CH, i * PB)  # causal edge in chunk
                            nc.tensor.matmul(
                                st_t[:, q_lo - hh * 2 * CH:(c - hh * 2 + 1) * CH],
                                _r(kt[jq][rowo:rowo + DH, i * PB:(i + 1) * PB]),
                                _r(qt[jq][rowo:rowo + DH, q_lo:(c + 1) * CH]),
                                start=True, stop=True,
                            )
                        # exp(scale * s + pad_bias) over this half's valid span;
                        # on the diagonal half start at the 128-granular edge
                        s0 = max(lo_c * CH, i * PB)
                        span = (hh + 1) * 2 * CH - s0
                        nc.scalar.activation(
                            ex_t[:, s0:s0 + span],
                            st_t[:, s0 - hh * 2 * CH:s0 - hh * 2 * CH + span],
                            mybir.ActivationFunctionType.Exp,
                            bias=pbias_sb[:, i:i + 1], scale=SCALE,
                        )
                    # zero q < k inside the 128-wide diagonal block
                    nc.gpsimd.affine_select(
                        out=ex_t[:, i * PB:(i + 1) * PB],
                        in_=ex_t[:, i * PB:(i + 1) * PB],
                        compare_op=mybir.AluOpType.is_ge, fill=0.0,
                        base=0, pattern=[[1, PB]],
                        channel_multiplier=-1,
                    )
                    # accumulate O^T (and denominators) for all valid chunks;
                    # the diagonal chunk reads only from the causal edge on
                    for c in range(NCH - 1, c0 - 1, -1):
                        if c == c0:
                            off = i * PB - c0 * CH
                            out_ap = oaug(c)[:, off:CH]
                            rhs = ex_t[:, i * PB:(c0 + 1) * CH]
                        else:
                            out_ap = oaug(c)
                            rhs = ex_t[:, c * CH:(c + 1) * CH]
                        nc.tensor.matmul(
                            out_ap[0:DH + 1, :],
                            vt[i][:, h * (DH + 1):(h + 1) * (DH + 1)],
                            rhs,
                            start=(i == 0), stop=(i == 4 * c + 3),
                        )
                        if i == 4 * c + 3:
                            # normalize attT rows = O^T * (1/denom). Copy the
                            # raw O and the denom row out first (releases the
                            # psum bank); the reciprocal/broadcast/multiply
                            # chain then runs off the PE critical path.
                            dst = stg_t[:, c * CH:(c + 1) * CH]
                            dn_t = rcp.tile([DH + 1, CH], F32R, tag="rc", bufs=2, name="dn_t")
                            nc.vector.tensor_copy(dst, oaug(c)[0:DH, :])
                            nc.vector.tensor_copy(
                                dn_t[DH:DH + 1, :], oaug(c)[DH:DH + 1, :])
                            dnp_t = rcp.tile([PB, NCH], F32R, tag="dnp", bufs=2, name="dnp_t")
                            nc.sync.dma_start(out=dnp_t[:], in_=dn_t[DH:DH + 1, :])
                            rcs_t = rcp.tile([PB, NCH], F32R, tag="rcs", bufs=2, name="rcs_t")
                            with nc.allow_low_precision(reason="fp32r pipeline"):
                                nc.vector.reciprocal(rcs_t[:], dnp_t[:])
                            rc2_t = rcp.tile([1, CH], F32R, tag="rc2", bufs=2, name="rc2_t")
                            nc.sync.dma_start(out=rc2_t[:], in_=rcs_t[:])
                            # reuse dn_t rows 0..63 as the broadcast target
                            nc.gpsimd.partition_broadcast(
                                dn_t[0:DH, :], rc2_t[0:1, :])
                            nc.vector.tensor_tensor(
                                dst, dst, dn_t[0:DH, :],
                                mybir.AluOpType.mult,
                            )
                nc.sync.dma_start(
                    out=attd[h * DH:(h + 1) * DH, :], in_=stg_t[:])

            # ---------------- output projection ----------------
            att_half = {}
            for j in range(NJT):
                for hh in range(2):
                    t = bigp.tile([PB, HS], F32R, tag=f"b{j * 2 + hh}", name=f"ah{j}_{hh}")
                    nc.sync.dma_start(
                        out=t[0:DH, :],
                        in_=attd[j * PB:j * PB + DH, hh * HS:(hh + 1) * HS])
                    nc.sync.dma_start(
                        out=t[DH:PB, :],
                        in_=attd[j * PB + DH:(j + 1) * PB, hh * HS:(hh + 1) * HS])
                    att_half[(j, hh)] = t
            for m in range(D // PB):
                for c in range(NCH):
                    acc = pair_tile("ABCD"[c % 4])[:, 0:CH]
                    for j in range(NJT):
                        nc.tensor.matmul(
                            acc,
                            _r(wol[j][:, m * PB:(m + 1) * PB]),
                            _r(att_half[(j, c // 2)][:, (c % 2) * CH:(c % 2 + 1) * CH]),
                            start=(j == 0), stop=(j == NJT - 1),
                        )
                    ost = rcp.tile([PB, CH], F32, tag="ost", bufs=3, name="ost")
                    nc.vector.tensor_copy(ost[:], acc)
                    nc.sync.dma_start(
                        out=outT[m * PB:(m + 1) * PB, c * CH:(c + 1) * CH],
                        in_=ost[:])


def build_module():
    nc = bacc.Bacc()
    xq = nc.declare_dram_parameter("xqT", [D, S], F32R, isOutput=False)
    xkv = nc.declare_dram_parameter("xkvT", [D, S], F32R, isOutput=False)
    wq = nc.declare_dram_parameter("wqT", [D, DG], F32R, isOutput=False)
    wk = nc.declare_dram_parameter("wkT", [D, DG], F32R, isOutput=False)
    wv = nc.declare_dram_parameter("wvT", [D, DG], F32R, isOutput=False)
    wo = nc.declare_dram_parameter("woT", [DG, D], F32R, isOutput=False)
    pb = nc.declare_dram_parameter("pbias", [S], F32, isOutput=False)
    outT = nc.declare_dram_parameter("outT", [D, S], F32, isOutput=True)
    _emit(nc, xq, xkv, wq, wk, wv, wo, pb, outT)
    nc.finalize()
    return nc


_NC = None


def _get_nc():
    global _NC
    if _NC is None:
        _NC = build_module()
    return _NC


def make_in_maps(q_raw, kv_raw, padding_mask, Wq, Wk, Wv, Wo):
    q_raw = np.asarray(q_raw, np.float32)
    kv_raw = np.asarray(kv_raw, np.float32)
    qT = np.ascontiguousarray(q_raw.transpose(0, 2, 1))
    kvT = np.ascontiguousarray(kv_raw.transpose(0, 2, 1))
    pbias = np.where(np.asarray(padding_mask) == 0, -1e9, 0.0).astype(np.float32)
    Wq, Wk, Wv, Wo = (np.asarray(w, np.float32) for w in (Wq, Wk, Wv, Wo))
    wqT = [np.ascontiguousarray(Wq[g * DG:(g + 1) * DG, :].T) for g in range(NG)]
    wkT = [np.ascontiguousarray(Wk[g * DG:(g + 1) * DG, :].T) for g in range(NG)]
    wvT = [np.ascontiguousarray(Wv[g * DG:(g + 1) * DG, :].T) for g in range(NG)]
    woT = [np.ascontiguousarray(Wo[:, g * DG:(g + 1) * DG].T) for g in range(NG)]
    in_maps = []
    for c in range(NG * B):
        b, g = divmod(c, NG)
        in_maps.append({
            "xqT": qT[b], "xkvT": kvT[b],
            "wqT": wqT[g], "wkT": wkT[g], "wvT": wvT[g], "woT": woT[g],
            "pbias": pbias[b],
        })
    return in_maps


def kernel(q_raw, kv_raw, padding_mask, Wq, Wk, Wv, Wo):
    from concourse.bass_utils import run_bass_kernel_spmd

    nc = _get_nc()
    in_maps = make_in_maps(q_raw, kv_raw, padding_mask, Wq, Wk, Wv, Wo)
    res = run_bass_kernel_spmd(nc, in_maps, core_ids=list(range(NG * B)))
    out = np.empty((B, S, D), np.float32)
    for b in range(B):
        out[b] = (res.results[NG * b]["outT"] + res.results[NG * b + 1]["outT"]).T
    return out

